# revision 16
# baseline (speedup 1.0000x reference)
"""Trainium2 Bass kernel for the Blurkernel problem.

Computes blur_kernel[1,1,K,K] = normalize(exp(-x^2/(2 s1^2)
- 2 rho x y/(2 s1 s2) - y^2/(2 s2^2))) for K=511 on TRN2 NeuronCores.

Strategy: the grid is K x K with coordinates -R..R (R=K//2).  With
rho == 0 (the case the harness generates) the Gaussian separates:
e[y, x] = exp(c*y^2) * exp(a*x^2), and the global sum factors as
Sy * Sx.  Each core computes, fully on-device:
  - iota x coords [128, K] (same values on every partition)
  - ex = Exp(a*x^2) with free-dim accum -> Sx per partition (all equal)
  - eyrow = Exp(c*x^2) with accum -> Sy per partition (all equal)
  - its own 128 rows' ey from a per-core y-coord input [128,1]
  - out = ex * (ey / (Sx*Sy))  -> one [128, K] tile, DMA'd out
Cores 0..ntiles-1 cover distinct row blocks; the host stacks them.
A general (rho != 0) path computes the full-grid row sums on every
core via iota y tiles and a cross-partition reduce.
"""

import math
import sys
import types

import numpy as np

N_CORES = 8
P = 128


def _install_ntff_shim():
    """Make run_bass_kernel_spmd(trace=True) under axon degrade gracefully
    (or work, when the axon .so supports it) even though this image's
    antenv package lacks the axon_hooks module."""
    if "antenv.axon_hooks" in sys.modules:
        return
    try:
        import antenv.axon_hooks  # noqa: F401
        return
    except ImportError:
        pass
    hook = None
    try:
        from trn_agent_boot.trn_boot import _ntff_profile_via_ctypes

        hook = _ntff_profile_via_ctypes("/opt/axon/libaxon_pjrt.so")
    except Exception:
        hook = None
    mod = types.ModuleType("antenv.axon_hooks")
    mod.get_axon_ntff_profile_hook = lambda: hook
    sys.modules["antenv.axon_hooks"] = mod


def _build_raw(a, c, K, ntiles):
    """Raw-Bass (no Tile) build of the separable fast path: shorter
    pre/postamble than the Tile version.  Requires that the coordinate
    pad cells underflow exp() to zero (checked by the caller)."""
    import concourse.bacc as bacc
    import concourse.mybir as mybir

    R = K // 2
    F = mybir.dt.float32
    EXP = mybir.ActivationFunctionType.Exp
    MUL = mybir.AluOpType.mult

    nc = bacc.Bacc(
        "TRN2", target_bir_lowering=False, debug=False, num_devices=N_CORES
    )
    ycoord = nc.dram_tensor("ycoord", [P, 1], F, kind="ExternalInput")
    out = nc.dram_tensor("out", [P, K], F, kind="ExternalOutput")

    from contextlib import ExitStack

    with ExitStack() as ctx:
        t4 = ctx.enter_context(nc.sbuf_tensor("t4", [P, ntiles], F))
        ones = ctx.enter_context(nc.sbuf_tensor("ones", [P, P], F))
        xi = ctx.enter_context(nc.sbuf_tensor("xi", [P, K], F))
        yc = ctx.enter_context(nc.sbuf_tensor("yc", [P, 1], F))
        sq4 = ctx.enter_context(nc.sbuf_tensor("sq4", [P, ntiles], F))
        q8 = ctx.enter_context(nc.sbuf_tensor("q8", [P, 2 * ntiles], F))
        xsq = ctx.enter_context(nc.sbuf_tensor("xsq", [P, K], F))
        cysq = ctx.enter_context(nc.sbuf_tensor("cysq", [P, 1], F))
        e8 = ctx.enter_context(nc.sbuf_tensor("e8", [P, 2 * ntiles], F))
        red2 = ctx.enter_context(nc.sbuf_tensor("red2", [P, 2], F))
        s = ctx.enter_context(nc.sbuf_tensor("s", [P, 1], F))
        inv = ctx.enter_context(nc.sbuf_tensor("inv", [P, 1], F))
        exw = ctx.enter_context(nc.sbuf_tensor("exw", [P, K], F))
        osb = ctx.enter_context(nc.sbuf_tensor("osb", [P, K], F))
        psumB = ctx.enter_context(nc.psum_tensor("psumB", [P, 2 * ntiles], F))
        s_pl = ctx.enter_context(nc.semaphore())
        s_in = ctx.enter_context(nc.semaphore())
        s_dve = ctx.enter_context(nc.semaphore())
        s_act = ctx.enter_context(nc.semaphore())
        s_pe = ctx.enter_context(nc.semaphore())
        s_out = ctx.enter_context(nc.semaphore())
        block = ctx.enter_context(nc.Block(no_gpsimd_drain=True))

        @block.gpsimd
        def _(gpsimd):
            nc.gpsimd.iota(
                t4[:], [[P, ntiles]], base=-R, channel_multiplier=1,
                allow_small_or_imprecise_dtypes=True,
            ).then_inc(s_pl)  # 1
            nc.gpsimd.memset(ones[:], 1.0).then_inc(s_pl)  # 2
            nc.gpsimd.iota(
                xi[:], [[1, K]], base=-R, channel_multiplier=0,
                allow_small_or_imprecise_dtypes=True,
            ).then_inc(s_pl)  # 3

        @block.sync
        def _(sync):
            sync.dma_start(yc[:], ycoord[:, :]).then_inc(s_in, 16)
            sync.wait_ge(s_dve, 9)
            sync.dma_start(out[:, :], osb[:]).then_inc(s_out, 16)

        @block.vector
        def _(vector):
            vector.wait_ge(s_pl, 1)
            nc.vector.tensor_mul(sq4[:], t4[:], t4[:]).then_inc(s_dve)  # 1
            # pre-scaled squares so one ACT op covers both exp groups
            vector.wait_ge(s_dve, 1)
            nc.vector.tensor_scalar_mul(
                q8[:, 0:ntiles], sq4[:], a
            ).then_inc(s_dve)  # 2
            nc.vector.tensor_scalar_mul(
                q8[:, ntiles : 2 * ntiles], sq4[:], c
            ).then_inc(s_dve)  # 3
            vector.wait_ge(s_pl, 3)
            nc.vector.tensor_mul(xsq[:], xi[:], xi[:]).then_inc(s_dve)  # 4
            vector.wait_ge(s_in, 16)
            nc.vector.scalar_tensor_tensor(
                cysq[:], yc[:], c, yc[:], op0=MUL, op1=MUL
            ).then_inc(s_dve)  # 5
            vector.wait_ge(s_pe, 1)
            nc.vector.tensor_reduce(
                red2[:],
                psumB[:, :].rearrange("p (g j) -> p g j", g=2),
                axis=mybir.AxisListType.X,
                op=mybir.AluOpType.add,
            ).then_inc(s_dve)  # 6
            # same-engine RAW hazards: DVE is deeply pipelined, wait for
            # the producer's completion sem before consuming
            vector.wait_ge(s_dve, 6)
            nc.vector.tensor_mul(
                s[:], red2[:, 0:1], red2[:, 1:2]
            ).then_inc(s_dve)  # 7
            vector.wait_ge(s_dve, 7)
            nc.vector.reciprocal(inv[:], s[:]).then_inc(s_dve)  # 8
            vector.wait_ge(s_dve, 8)
            vector.wait_ge(s_act, 2)
            nc.vector.tensor_scalar_mul(
                osb[:], exw[:], inv[:]
            ).then_inc(s_dve)  # 9

        @block.scalar
        def _(scalar):
            scalar.wait_ge(s_dve, 3)
            nc.scalar.activation(e8[:, :], q8[:, :], EXP).then_inc(s_act)  # 1
            scalar.wait_ge(s_dve, 5)
            nc.scalar.activation(
                exw[:], xsq[:], EXP, bias=cysq[:], scale=a
            ).then_inc(s_act)  # 2

        @block.tensor
        def _(tensor):
            tensor.wait_ge(s_act, 1)
            tensor.wait_ge(s_pl, 2)
            nc.tensor.matmul(
                psumB[:], ones[:], e8[:], start=True, stop=True
            ).then_inc(s_pe)  # 1

    nc.compile()
    return nc


def _build(a, c, b, K, ntiles, use_rho):
    """Trace and compile the Bass kernel. a, c, b are f32 immediates."""
    import concourse.bacc as bacc
    import concourse.mybir as mybir
    import concourse.tile as tile

    R = K // 2
    F = mybir.dt.float32
    EXP = mybir.ActivationFunctionType.Exp

    nc = bacc.Bacc(
        "TRN2", target_bir_lowering=False, debug=False, num_devices=N_CORES
    )
    ycoord = nc.dram_tensor("ycoord", [P, 1], F, kind="ExternalInput")
    out = nc.dram_tensor("out", [P, K], F, kind="ExternalOutput")

    with tile.TileContext(nc) as tc:
        with (
            tc.tile_pool(name="pool", bufs=1) as pool,
            tc.tile_pool(name="psum", bufs=1, space="PSUM") as psum,
        ):
            if not use_rho:
                # --- separable fast path ---
                # out[p, f] = Exp(a*x_f^2 + c*y_p^2) / (Sx*Sy), with the
                # row term folded into the wide exp's per-partition bias.

                # Narrow coord tile: col j, partition p -> -R + 128j + p.
                # Covers every coordinate value used by both x and y.
                t4 = pool.tile([P, ntiles], F)
                nc.gpsimd.iota(
                    t4[:], [[P, ntiles]], base=-R, channel_multiplier=1,
                    allow_small_or_imprecise_dtypes=True,
                )
                ones = pool.tile([P, P], F)
                nc.gpsimd.memset(ones[:], 1.0)
                # wide x coords -R..R along the free dim, same per partition
                xi = pool.tile([P, K], F)
                nc.gpsimd.iota(
                    xi[:], [[1, K]], base=-R, channel_multiplier=0,
                    allow_small_or_imprecise_dtypes=True,
                )

                # this core's 128 y coords (data differs per core)
                yc = pool.tile([P, 1], F)
                nc.sync.dma_start(yc[:], ycoord[:, :])

                sq4 = pool.tile([P, ntiles], F)
                nc.vector.tensor_mul(sq4[:], t4[:], t4[:])

                # wide x^2 in two column chunks so the exp can chase it
                H = (K + 1) // 2
                xsq = pool.tile([P, K], F)
                nc.vector.tensor_mul(xsq[:, 0:H], xi[:, 0:H], xi[:, 0:H])
                # c*y^2 in one op: (yc*c)*yc
                cysq = pool.tile([P, 1], F)
                nc.vector.scalar_tensor_tensor(
                    cysq[:], yc[:], c, yc[:],
                    op0=mybir.AluOpType.mult, op1=mybir.AluOpType.mult,
                )
                nc.vector.tensor_mul(xsq[:, H:K], xi[:, H:K], xi[:, H:K])

                # e8: cols [0, ntiles) = exp(a*v^2), [ntiles, 2*ntiles) =
                # exp(c*v^2) over the full coordinate set
                e8 = pool.tile([P, 2 * ntiles], F)
                nc.scalar.activation(e8[:, 0:ntiles], sq4[:], EXP, scale=a)
                nc.scalar.activation(
                    e8[:, ntiles : 2 * ntiles], sq4[:], EXP, scale=c
                )
                pad = ntiles * P - K
                if pad > 0 and (R + 1) ** 2 * min(abs(a), abs(c)) < 104.0:
                    # pad coords don't underflow to 0; zero them explicitly
                    nc.vector.memset(e8[P - pad :, ntiles - 1 : ntiles], 0.0)
                    nc.vector.memset(
                        e8[P - pad :, 2 * ntiles - 1 : 2 * ntiles], 0.0
                    )

                # column sums of e8 broadcast to every partition with one
                # ones[128,128] matmul: psumB[p, j] = sum_k e8[k, j]
                psumB = psum.tile([P, 2 * ntiles], F)
                nc.tensor.matmul(
                    psumB[:], ones[:], e8[:], start=True, stop=True
                )
                # (Sx, Sy) per partition: free-dim sums of the two groups
                red2 = pool.tile([P, 2], F)
                nc.vector.tensor_reduce(
                    red2[:],
                    psumB[:, :].rearrange("p (g j) -> p g j", g=2),
                    axis=mybir.AxisListType.X,
                    op=mybir.AluOpType.add,
                )
                s = pool.tile([P, 1], F)
                nc.vector.tensor_mul(s[:], red2[:, 0:1], red2[:, 1:2])
                inv = pool.tile([P, 1], F)
                nc.vector.reciprocal(inv[:], s[:])

                # wide pass: exp in two chunks, then one scale + store
                exw = pool.tile([P, K], F)
                nc.scalar.activation(
                    exw[:, 0:H], xsq[:, 0:H], EXP, bias=cysq[:], scale=a
                )
                nc.scalar.activation(
                    exw[:, H:K], xsq[:, H:K], EXP, bias=cysq[:], scale=a
                )
                osb = pool.tile([P, K], F)
                nc.vector.tensor_scalar_mul(osb[:], exw[:], inv[:])
                nc.sync.dma_start(out[:, :], osb[:])
            else:
                # General path: log_k = a*x^2 + (b*y)*x + c*y^2.
                # Full-grid row sums on every core via iota y tiles.
                xi = pool.tile([P, K], F)
                nc.gpsimd.iota(
                    xi[:], [[1, K]], base=-R, channel_multiplier=0,
                    allow_small_or_imprecise_dtypes=True,
                )
                xsq = pool.tile([P, K], F)
                nc.vector.tensor_mul(xsq[:], xi[:], xi[:])
                yc = pool.tile([P, 1], F)
                nc.sync.dma_start(yc[:], ycoord[:, :])
                ysq = pool.tile([P, 1], F)
                nc.vector.tensor_mul(ysq[:], yc[:], yc[:])
                rs_tot = pool.tile([P, 1], F)
                for t in range(ntiles):
                    yt = pool.tile([P, 1], F, tag=f"yt{t}")
                    nc.gpsimd.iota(
                        yt[:], [[0, 1]], base=t * P - R, channel_multiplier=1,
                        allow_small_or_imprecise_dtypes=True,
                    )
                    ysqt = pool.tile([P, 1], F, tag=f"ysqt{t}")
                    nc.vector.tensor_mul(ysqt[:], yt[:], yt[:])
                    cyt = pool.tile([P, 1], F, tag=f"cyt{t}")
                    nc.scalar.mul(cyt[:], ysqt[:], c)
                    byt = pool.tile([P, 1], F, tag=f"byt{t}")
                    nc.scalar.mul(byt[:], yt[:], b)
                    v = pool.tile([P, K], F, tag=f"v{t}")
                    nc.vector.tensor_scalar_mul(v[:], xi[:], byt[:])
                    v2 = pool.tile([P, K], F, tag=f"v2{t}")
                    nc.vector.scalar_tensor_tensor(
                        v2[:], xsq[:], a, v[:],
                        op0=mybir.AluOpType.mult, op1=mybir.AluOpType.add,
                    )
                    et = pool.tile([P, K], F, tag=f"et{t}")
                    rst = pool.tile([P, 1], F, tag=f"rst{t}")
                    nc.scalar.activation(
                        et[:], v2[:], EXP, bias=cyt[:], accum_out=rst[:]
                    )
                    pad = ntiles * P - K
                    if t == ntiles - 1 and pad > 0:
                        nc.vector.memset(rst[P - pad :, :], 0.0)
                    if t == 0:
                        nc.vector.tensor_copy(rs_tot[:], rst[:])
                    else:
                        nc.vector.tensor_add(rs_tot[:], rs_tot[:], rst[:])
                # cross-partition total, broadcast to all partitions
                stot = pool.tile([P, 1], F)
                nc.gpsimd.partition_all_reduce(
                    stot[:], rs_tot[:], op=mybir.AluOpType.add
                )
                inv = pool.tile([P, 1], F)
                nc.vector.reciprocal(inv[:], stot[:])

                # this core's own rows from the ycoord input
                cy = pool.tile([P, 1], F)
                nc.scalar.mul(cy[:], ysq[:], c)
                by = pool.tile([P, 1], F)
                nc.scalar.mul(by[:], yc[:], b)
                v = pool.tile([P, K], F)
                nc.vector.tensor_scalar_mul(v[:], xi[:], by[:])
                v2 = pool.tile([P, K], F)
                nc.vector.scalar_tensor_tensor(
                    v2[:], xsq[:], a, v[:],
                    op0=mybir.AluOpType.mult, op1=mybir.AluOpType.add,
                )
                e = pool.tile([P, K], F)
                nc.scalar.activation(e[:], v2[:], EXP, bias=cy[:])
                osb = pool.tile([P, K], F)
                nc.vector.tensor_scalar_mul(osb[:], e[:], inv[:])
                nc.sync.dma_start(out[:, :], osb[:])

    nc.compile()
    return nc


LAST_RESULTS = None


def kernel(sigma1, sigma2, rho, kernel_size):
    _install_ntff_shim()
    from concourse.bass_utils import run_bass_kernel_spmd

    global LAST_RESULTS

    s1 = float(np.asarray(sigma1, dtype=np.float64).reshape(-1)[0])
    s2 = float(np.asarray(sigma2, dtype=np.float64).reshape(-1)[0])
    rv = float(np.asarray(rho, dtype=np.float64).reshape(-1)[0])
    K = int(np.asarray(kernel_size).reshape(-1)[0])
    R = K // 2
    ntiles = max(1, math.ceil(K / P))
    assert ntiles <= N_CORES, "kernel only supports K <= 1024"

    # launch constants (specialized per call; immediates in the kernel)
    a = float(np.float32(-1.0 / (2.0 * s1 * s1)))
    c = float(np.float32(-1.0 / (2.0 * s2 * s2)))
    b = float(np.float32(-rv / (s1 * s2)))
    use_rho = rv != 0.0

    pad = ntiles * P - K
    pad_underflows = pad == 0 or (R + 1) ** 2 * min(abs(a), abs(c)) >= 104.0
    if not use_rho and pad_underflows:
        nc = _build_raw(a, c, K, ntiles)
    else:
        nc = _build(a, c, b, K, ntiles, use_rho)

    in_maps = []
    for core in range(N_CORES):
        t = min(core, ntiles - 1)
        yvals = (np.arange(P, dtype=np.float32) + np.float32(t * P - R))[
            :, None
        ]
        in_maps.append({"ycoord": yvals})

    res = run_bass_kernel_spmd(nc, in_maps, core_ids=list(range(N_CORES)))
    LAST_RESULTS = res

    rows = np.vstack([res.results[t]["out"] for t in range(ntiles)])[:K]
    return rows.reshape(1, 1, K, K).astype(np.float32, copy=False)
